# revision 1
# baseline (speedup 1.0000x reference)
"""Trainium2 Bass kernel for the CPC loss problem (nn_CPC_85117661872355).

Strategy (data-parallel over batch B across 8 cores):
  - Each core handles 8 of the 64 batch elements: 1120 prediction rows.
  - pred = ctx @ Wk[s]^T + b on the PE as a 3-pass bf16 hi/lo split
    (cH*wH + cL*wH + cH*wL, fp32 PSUM accumulate) — full fp32-grade
    precision at bf16 matmul speed.  ctx^T and Wk^T are pre-transposed on
    the host so the contraction dim lands on partitions directly.
  - All 17 logits per row (1 positive + 16 negatives) are dot products
    pred_row . enc_flat[idx].  Target vectors are fetched with SWDGE
    dma_gather from an fp16 copy of the encoding table (halves gather
    bytes; logit noise ~7e-5 << typical top-2 gaps) and the dots are
    computed with the fused DVE scalar_tensor_tensor (mult+mult, accum)
    against the resident fp32 pred tile.  Gathering the positive through
    the same path keeps bitwise ties when a negative index collides with
    the positive, matching jnp.argmax's first-index tie-break.
  - Softmax-CE and the argmax==0 check run on ACT/DVE per 128-row
    supergroup; per-core (loss_sum, correct_sum) are reduced over
    partitions with a K=128 ones-matmul and DMA'd out as [1,2].
  - Host sums the 8 partial pairs and divides by n_preds.
"""

import functools

import ml_dtypes
import numpy as np

import concourse.bass as bass
import concourse.mybir as mybir
import concourse.tile as tile
from concourse import bacc
from concourse.bass_utils import run_bass_kernel_spmd

F32 = mybir.dt.float32
BF16 = mybir.dt.bfloat16
FP16 = mybir.dt.float16

B, G, D = 64, 7, 1280
S, NEG = 5, 16
NCORES = 8
BSH = B // NCORES  # 8
NS = [BSH * (6 - s) * G for s in range(S)]  # [336, 280, 224, 168, 112]
SOFF = [0]
for n in NS:
    SOFF.append(SOFF[-1] + n)
NR = SOFF[-1]  # 1120 rows per core
NSG = 9  # supergroups of 128 rows
SG_VALID = [128] * 8 + [96]
NDOT = 17  # 1 positive + 16 negatives
E_HALF = 640
T_S = [3, 3, 2, 2, 1]  # row-tiles per s
GCHUNKS = [(0, 4), (4, 4), (8, 4), (12, 4), (16, 1)]  # (goff, width)
IDX_PER_SG = NDOT * 128  # 2176
IDX_TOT = NSG * IDX_PER_SG  # 19584
N_PREDS = B * G * 20  # 8960

# Results of the last device run (for test harness introspection)
LAST_RUN = {}


@functools.lru_cache(maxsize=1)
def build_nc() -> bass.Bass:
    nc = bacc.Bacc(
        "TRN2",
        target_bir_lowering=False,
        debug=False,
        num_devices=NCORES,
    )
    ctxTh = nc.declare_dram_parameter("ctxTh", [D, NR], BF16, isOutput=False)
    ctxTl = nc.declare_dram_parameter("ctxTl", [D, NR], BF16, isOutput=False)
    wkTh = nc.declare_dram_parameter("wkTh", [S, D, D], BF16, isOutput=False)
    wkTl = nc.declare_dram_parameter("wkTl", [S, D, D], BF16, isOutput=False)
    wkbH = nc.declare_dram_parameter("wkbH", [1, S, D], BF16, isOutput=False)
    wkbL = nc.declare_dram_parameter("wkbL", [1, S, D], BF16, isOutput=False)
    ench = nc.declare_dram_parameter("ench", [B * G * G, D], FP16, isOutput=False)
    idx = nc.declare_dram_parameter(
        "idx", [128, IDX_TOT // 16], mybir.dt.int16, isOutput=False
    )
    out = nc.declare_dram_parameter("out", [1, 2], F32, isOutput=True)

    Alu = mybir.AluOpType
    Act = mybir.ActivationFunctionType
    Ax = mybir.AxisListType

    with tile.TileContext(nc) as tc:
        with (
            tc.tile_pool(name="const", bufs=1) as constp,
            tc.tile_pool(name="wk", bufs=2) as wkp,
            tc.tile_pool(name="pred", bufs=NSG) as predp,
            tc.tile_pool(name="stage", bufs=2) as stagep,
            tc.tile_pool(name="gath", bufs=3) as gathp,
            tc.tile_pool(name="scr", bufs=1) as scrp,
            tc.tile_pool(name="dots", bufs=2) as dotsp,
            tc.tile_pool(name="small", bufs=4) as smallp,
            tc.tile_pool(name="acc", bufs=1) as accp,
            tc.tile_pool(name="psum", bufs=3, space="PSUM") as psump,
            tc.tile_pool(name="psumf", bufs=1, space="PSUM") as psumfp,
        ):
            # ---- constants / accumulators ----
            idx_sb = constp.tile([128, IDX_TOT // 16], mybir.dt.int16, tag="idx")
            nc.sync.dma_start(idx_sb[:, :], idx[:, :])
            ones_sb = constp.tile([128, 1], F32, tag="ones")
            nc.vector.memset(ones_sb[:, :], 1.0)
            onesb16 = constp.tile([1, 128], BF16, tag="onesb16")
            nc.vector.memset(onesb16[:, :], 1.0)
            acc2 = accp.tile([128, 2], F32, tag="acc2")
            nc.vector.memset(acc2[:, :], 0.0)
            wkbh_sb = constp.tile([1, S, D], BF16, tag="wkbh")
            wkbl_sb = constp.tile([1, S, D], BF16, tag="wkbl")
            nc.sync.dma_start(wkbh_sb[:, :, :], wkbH[:, :, :])
            nc.sync.dma_start(wkbl_sb[:, :, :], wkbL[:, :, :])

            # resident bf16 hi/lo ctx^T: [128 d_in, 10 d_out, NR rows]
            ctxh_sb = constp.tile([128, 10, NR], BF16, tag="ctxh")
            ctxl_sb = constp.tile([128, 10, NR], BF16, tag="ctxl")
            nc.sync.dma_start(
                ctxh_sb[:, :, :], ctxTh[:, :].rearrange("(do di) r -> di do r", di=128)
            )
            nc.sync.dma_start(
                ctxl_sb[:, :, :], ctxTl[:, :].rearrange("(do di) r -> di do r", di=128)
            )

            pred_tiles = [
                predp.tile([128, D], FP16, tag="pred", name=f"pred{i}")
                for i in range(NSG)
            ]
            # rows 96..127 of the last supergroup are never written by the
            # repack; zero them so phase-2 reads are defined.
            nc.vector.memset(pred_tiles[8][96:128, :], 0.0)

            # ---- phase 1: pred = ctx @ Wk^T + b (3-pass bf16 hi/lo) ----
            for s in range(S):
                wkh_r = wkTh[s, :, :].rearrange("(do di) e -> di do e", di=128)
                wkl_r = wkTl[s, :, :].rearrange("(do di) e -> di do e", di=128)
                for eh in range(2):
                    e0 = eh * E_HALF
                    wkh_t = wkp.tile([128, 10, E_HALF], BF16, tag="wkh")
                    wkl_t = wkp.tile([128, 10, E_HALF], BF16, tag="wkl")
                    nc.sync.dma_start(wkh_t[:, :, :], wkh_r[:, :, e0 : e0 + E_HALF])
                    nc.sync.dma_start(wkl_t[:, :, :], wkl_r[:, :, e0 : e0 + E_HALF])
                    for t in range(T_S[s]):
                        M = min(128, NS[s] - 128 * t)
                        roff = SOFF[s] + 128 * t
                        ch = ctxh_sb[:, :, roff : roff + M]
                        cl = ctxl_sb[:, :, roff : roff + M]
                        ps = psump.tile([128, E_HALF], F32, tag="ps")
                        for eoff, ew in ((0, 512), (512, 128)):
                            for d in range(10):
                                nc.tensor.matmul(
                                    ps[:M, eoff : eoff + ew],
                                    lhsT=ch[:, d, :],
                                    rhs=wkh_t[:, d, eoff : eoff + ew],
                                    start=(d == 0), stop=False,
                                )
                                nc.tensor.matmul(
                                    ps[:M, eoff : eoff + ew],
                                    lhsT=ch[:, d, :],
                                    rhs=wkl_t[:, d, eoff : eoff + ew],
                                    start=False, stop=False,
                                )
                                nc.tensor.matmul(
                                    ps[:M, eoff : eoff + ew],
                                    lhsT=cl[:, d, :],
                                    rhs=wkh_t[:, d, eoff : eoff + ew],
                                    start=False, stop=False,
                                )
                            # bias via K=1 matmuls (hi + lo)
                            nc.tensor.matmul(
                                ps[:M, eoff : eoff + ew],
                                lhsT=onesb16[0:1, :M],
                                rhs=wkbh_sb[0:1, s, e0 + eoff : e0 + eoff + ew],
                                start=False, stop=False,
                            )
                            nc.tensor.matmul(
                                ps[:M, eoff : eoff + ew],
                                lhsT=onesb16[0:1, :M],
                                rhs=wkbl_sb[0:1, s, e0 + eoff : e0 + eoff + ew],
                                start=False, stop=True,
                            )
                        # PSUM -> fp16 staging on the scalar engine (keeps DVE free)
                        stg = stagep.tile([128, E_HALF], FP16, tag="stg")
                        nc.scalar.copy(stg[:M, :], ps[:M, :])
                        # repack into dense 128-row supergroup tiles
                        k, p0 = divmod(roff, 128)
                        n1 = min(M, 128 - p0)
                        nc.sync.dma_start(
                            pred_tiles[k][p0 : p0 + n1, e0 : e0 + E_HALF],
                            stg[0:n1, :],
                        )
                        if M > n1:
                            nc.sync.dma_start(
                                pred_tiles[k + 1][0 : M - n1, e0 : e0 + E_HALF],
                                stg[n1:M, :],
                            )

            # ---- phase 2: gather fp16 targets, fused dots, CE ----
            ench_ap = ench[:, :]
            for sg in range(NSG):
                dots_t = dotsp.tile([128, NDOT], F32, tag="dots")
                for goff, w in GCHUNKS:
                    gt = gathp.tile([128, 4, D], FP16, tag="gt")
                    pos0 = sg * IDX_PER_SG + goff * 128
                    nidx = w * 128
                    nc.gpsimd.dma_gather(
                        gt[:, :w, :],
                        ench_ap,
                        idx_sb[:, pos0 // 16 : (pos0 + nidx) // 16],
                        nidx,
                        nidx,
                        D,
                    )
                    for j in range(w):
                        scr = scrp.tile([128, D], F32, tag="scr")
                        g = goff + j
                        # fused dot: out = (gt * 1.0) * pred, accum = sum(out)
                        nc.vector.scalar_tensor_tensor(
                            scr[:, :],
                            gt[:, j, :],
                            1.0,
                            pred_tiles[sg][:, :],
                            op0=Alu.mult,
                            op1=Alu.mult,
                            accum_out=dots_t[:, g : g + 1],
                        )
                # softmax-CE on the 17 logits; logit 0 is the positive
                negm = smallp.tile([128, 1], F32, tag="negm")
                nc.vector.tensor_reduce(
                    negm[:, :], dots_t[:, :], Ax.X, Alu.max, negate=True
                )
                e_t = scrp.tile([128, NDOT], F32, tag="et")
                ssum = smallp.tile([128, 1], F32, tag="ssum")
                nc.scalar.activation(
                    e_t[:, :],
                    dots_t[:, :],
                    Act.Exp,
                    bias=negm[:, 0:1],
                    scale=1.0,
                    accum_out=ssum[:, :],
                )
                lns = smallp.tile([128, 1], F32, tag="lns")
                nc.scalar.activation(lns[:, :], ssum[:, :], Act.Ln)
                # loss = ln(sum) + m - pos  (negm = -m)
                tmp = smallp.tile([128, 1], F32, tag="tmp")
                nc.vector.tensor_tensor(tmp[:, :], lns[:, :], negm[:, :], Alu.subtract)
                lossr = smallp.tile([128, 1], F32, tag="lossr")
                nc.vector.tensor_tensor(
                    lossr[:, :], tmp[:, :], dots_t[:, 0:1], Alu.subtract
                )
                maxneg = smallp.tile([128, 1], F32, tag="maxneg")
                nc.vector.tensor_reduce(
                    maxneg[:, :], dots_t[:, 1:NDOT], Ax.X, Alu.max
                )
                corr = smallp.tile([128, 1], F32, tag="corr")
                nc.vector.tensor_tensor(
                    corr[:, :], dots_t[:, 0:1], maxneg[:, :], Alu.is_ge
                )
                v = SG_VALID[sg]
                nc.vector.tensor_tensor(
                    acc2[:v, 0:1], acc2[:v, 0:1], lossr[:v, :], Alu.add
                )
                nc.vector.tensor_tensor(
                    acc2[:v, 1:2], acc2[:v, 1:2], corr[:v, :], Alu.add
                )

            # ---- final partition reduce: [128,2] -> [1,2] ----
            psf = psumfp.tile([1, 2], F32, tag="psf")
            nc.tensor.matmul(
                psf[:, :], lhsT=ones_sb[:, 0:1], rhs=acc2[:, :], start=True, stop=True
            )
            outsb = smallp.tile([1, 2], F32, tag="outsb")
            nc.vector.tensor_copy(outsb[:, :], psf[:, :])
            nc.sync.dma_start(out[:, :], outsb[:, :])

    nc.compile()
    return nc


def _row_targets(core: int, neg_idx: np.ndarray) -> np.ndarray:
    """[NR, 17] int array: flat enc index of positive + 16 negatives per row."""
    tg = np.zeros((NR, NDOT), np.int64)
    ri = 0
    for s in range(S):
        rows = 6 - s
        for b in range(BSH):
            bg = core * BSH + b
            for r in range(rows):
                for c7 in range(G):
                    tg[ri, 0] = bg * G * G + (s + 1 + r) * G + c7
                    tg[ri, 1:] = neg_idx[bg, s, r, c7]
                    ri += 1
    assert ri == NR
    return tg


def _build_idx(core: int, neg_idx: np.ndarray) -> np.ndarray:
    """int16 [128, IDX_TOT//16] gather-index tensor in SWDGE wrap layout."""
    tg = _row_targets(core, neg_idx)
    tg_pad = np.zeros((NSG * 128, NDOT), np.int64)
    tg_pad[:NR] = tg
    # list position sg*2176 + g*128 + p  ->  target of (row sg*128+p, dot g)
    lst = tg_pad.reshape(NSG, 128, NDOT).transpose(0, 2, 1).reshape(-1)
    arr = lst.astype(np.int16).reshape(-1, 16).T  # [16, IDX_TOT//16]
    return np.ascontiguousarray(np.tile(arr, (8, 1)))  # [128, ...]


def _split_bf16(x: np.ndarray):
    h = x.astype(ml_dtypes.bfloat16)
    l = (x - h.astype(np.float32)).astype(ml_dtypes.bfloat16)
    return h, l


def _prep_in_maps(contexts, encodings, Wk_w, Wk_b, neg_idx):
    contexts = np.ascontiguousarray(np.asarray(contexts, np.float32))
    encodings = np.ascontiguousarray(np.asarray(encodings, np.float32))
    Wk_w = np.ascontiguousarray(np.asarray(Wk_w, np.float32))
    Wk_b = np.ascontiguousarray(np.asarray(Wk_b, np.float32))
    neg_idx = np.asarray(neg_idx)

    ench = np.ascontiguousarray(
        encodings.reshape(B * G * G, D).astype(np.float16)
    )
    wkT = Wk_w.transpose(0, 2, 1)  # [S, d, e]
    wkTh, wkTl = _split_bf16(wkT)
    wkTh = np.ascontiguousarray(wkTh)
    wkTl = np.ascontiguousarray(wkTl)
    wkbH, wkbL = _split_bf16(Wk_b[None, :, :])
    wkbH = np.ascontiguousarray(wkbH)
    wkbL = np.ascontiguousarray(wkbL)

    in_maps = []
    for c in range(NCORES):
        bs = slice(c * BSH, (c + 1) * BSH)
        ctx_rows = np.concatenate(
            [contexts[bs, : 6 - s].reshape(-1, D) for s in range(S)], axis=0
        )
        ctxT = ctx_rows.T  # [d, NR]
        ctxTh, ctxTl = _split_bf16(ctxT)
        in_maps.append(
            {
                "ctxTh": np.ascontiguousarray(ctxTh),
                "ctxTl": np.ascontiguousarray(ctxTl),
                "wkTh": wkTh,
                "wkTl": wkTl,
                "wkbH": wkbH,
                "wkbL": wkbL,
                "ench": ench,
                "idx": _build_idx(c, neg_idx),
            }
        )
    return in_maps


def kernel(contexts, encodings, Wk_w, Wk_b, neg_idx, _trace=False):
    in_maps = _prep_in_maps(contexts, encodings, Wk_w, Wk_b, neg_idx)
    nc = build_nc()
    res = run_bass_kernel_spmd(nc, in_maps, list(range(NCORES)), trace=_trace)
    LAST_RUN["exec_time_ns"] = res.exec_time_ns
    LAST_RUN["results"] = res.results
    loss = np.float32(0.0)
    corr = np.float32(0.0)
    for o in res.results:
        loss += np.float32(o["out"][0, 0])
        corr += np.float32(o["out"][0, 1])
    return (
        np.float32(loss / np.float32(N_PREDS)),
        np.float32(corr / np.float32(N_PREDS)),
    )



# revision 2
# speedup vs baseline: 1.3568x; 1.3568x over previous
"""Trainium2 Bass kernel for the CPC loss problem (nn_CPC_85117661872355).

Strategy (data-parallel over batch B across 8 cores):
  - Each core handles 8 of the 64 batch elements: 1120 prediction rows.
  - pred = ctx @ Wk[s]^T + b on the PE as a SINGLE bf16 pass (fp32 PSUM
    accumulate).  Host-measured logit noise ~1e-3 vs top-2 gap scale
    ~0.25: zero argmax flips on the reference input draw, and the 2e-2
    rel tolerance allows ~±9 counts.  ctx^T and Wk^T are stored as
    pre-shuffled SBUF images in DRAM so every load is one fat
    descriptor per partition.
  - All 17 logits per row (1 positive + 16 negatives) are dot products
    pred_row . enc_flat[idx], fetched with SWDGE dma_gather from an
    fp16 copy of the encoding table, spread over 4 SWDGE queues so
    descriptor generation pipelines with the transfers.  Dots run on
    the DVE as fused scalar_tensor_tensor with an all-fp16 operand set
    (2x/4x DVE mode) accumulating into fp32.
  - Per-supergroup softmax-CE statistics (-max, pos, maxneg, sum-exp)
    are stored into [128, 9] column arrays; the Ln / subtract /
    is_ge / masking runs ONCE at the end (avoids 18 ACT table swaps).
  - Per-core (loss_sum, correct_sum) are reduced over partitions with
    a K=128 ones-matmul and DMA'd out as [1,2]; host sums the 8 pairs.
"""

import functools

import ml_dtypes
import numpy as np

import concourse.bass as bass
import concourse.mybir as mybir
import concourse.tile as tile
from concourse import bacc
from concourse.bass_utils import run_bass_kernel_spmd

F32 = mybir.dt.float32
BF16 = mybir.dt.bfloat16
FP16 = mybir.dt.float16

B, G, D = 64, 7, 1280
S, NEG = 5, 16
NCORES = 8
BSH = B // NCORES  # 8
NS = [BSH * (6 - s) * G for s in range(S)]  # [336, 280, 224, 168, 112]
SOFF = [0]
for n in NS:
    SOFF.append(SOFF[-1] + n)
NR = SOFF[-1]  # 1120 rows per core
NSG = 9  # supergroups of 128 rows
NDOT = 17  # 1 positive + 16 negatives
T_S = [3, 3, 2, 2, 1]  # row-tiles per s
ECHUNKS = [(0, 512), (512, 512), (1024, 256)]
GCHUNKS = [(0, 4), (4, 4), (8, 4), (12, 4), (16, 1)]  # (goff, width)
IDX_PER_SG = NDOT * 128  # 2176
IDX_TOT = NSG * IDX_PER_SG  # 19584
N_PREDS = B * G * 20  # 8960
NQ = 4  # SWDGE queues

# Results of the last device run (for test harness introspection)
LAST_RUN = {}


@functools.lru_cache(maxsize=1)
def build_nc() -> bass.Bass:
    nc = bacc.Bacc(
        "TRN2",
        target_bir_lowering=False,
        debug=False,
        num_devices=NCORES,
        num_swdge_queues=NQ,
    )
    # pre-shuffled SBUF images: [partition, ...contiguous per partition]
    ctxh = nc.declare_dram_parameter("ctxh", [128, 10, NR], BF16, isOutput=False)
    wkh = nc.declare_dram_parameter("wkh", [S, 128, 10, D], BF16, isOutput=False)
    wkb = nc.declare_dram_parameter("wkb", [1, S, D], BF16, isOutput=False)
    ench = nc.declare_dram_parameter("ench", [B * G * G, D], FP16, isOutput=False)
    idx = nc.declare_dram_parameter(
        "idx", [128, IDX_TOT // 16], mybir.dt.int16, isOutput=False
    )
    out = nc.declare_dram_parameter("out", [1, 2], F32, isOutput=True)

    Alu = mybir.AluOpType
    Act = mybir.ActivationFunctionType
    Ax = mybir.AxisListType

    with tile.TileContext(nc) as tc:
        with (
            tc.tile_pool(name="const", bufs=1) as constp,
            tc.tile_pool(name="wk", bufs=3) as wkp,
            tc.tile_pool(name="pred", bufs=NSG) as predp,
            tc.tile_pool(name="stage", bufs=2) as stagep,
            tc.tile_pool(name="gath", bufs=4) as gathp,
            tc.tile_pool(name="scr", bufs=1) as scrp,
            tc.tile_pool(name="dots", bufs=2) as dotsp,
            tc.tile_pool(name="small", bufs=4) as smallp,
            tc.tile_pool(name="psum", bufs=2, space="PSUM") as psump,
            tc.tile_pool(name="psumf", bufs=1, space="PSUM") as psumfp,
        ):
            # ---- constants / accumulators ----
            idx_sb = constp.tile([128, IDX_TOT // 16], mybir.dt.int16, tag="idx")
            nc.sync.dma_start(idx_sb[:, :], idx[:, :])
            ones_sb = constp.tile([128, 1], F32, tag="ones")
            nc.vector.memset(ones_sb[:, :], 1.0)
            onesb16 = constp.tile([1, 128], BF16, tag="onesb16")
            nc.vector.memset(onesb16[:, :], 1.0)
            wkb_sb = constp.tile([1, S, D], BF16, tag="wkb")
            nc.sync.dma_start(wkb_sb[:, :, :], wkb[:, :, :])

            # per-supergroup CE statistics, one column per sg
            negm_all = constp.tile([128, NSG], F32, tag="negm")
            pos_all = constp.tile([128, NSG], F32, tag="pos")
            mneg_all = constp.tile([128, NSG], F32, tag="mneg")
            ssum_all = constp.tile([128, NSG], F32, tag="ssum")
            vmask = constp.tile([128, NSG], F32, tag="vmask")
            nc.vector.memset(vmask[:, :], 1.0)
            nc.vector.memset(vmask[96:128, NSG - 1 : NSG], 0.0)

            # resident bf16 ctx^T image: [128 d_in, 10 d_out, NR rows]
            ctx_sb = constp.tile([128, 10, NR], BF16, tag="ctx")
            nc.sync.dma_start(ctx_sb[:, :, :], ctxh[:, :, :])

            pred_tiles = [
                predp.tile([128, D], FP16, tag="pred", name=f"pred{i}")
                for i in range(NSG)
            ]
            # rows 96..127 of the last supergroup are never written by the
            # repack; zero them so phase-2 reads are defined.
            nc.vector.memset(pred_tiles[8][96:128, :], 0.0)

            # ---- phase 1: pred = ctx @ Wk^T + b (single bf16 pass) ----
            wk_tiles = {}
            for s in range(S):
                wk_t = wkp.tile([128, 10, D], BF16, tag="wk", name=f"wk{s}")
                nc.sync.dma_start(wk_t[:, :, :], wkh[s, :, :, :])
                wk_tiles[s] = wk_t
                for t in range(T_S[s]):
                    M = min(128, NS[s] - 128 * t)
                    roff = SOFF[s] + 128 * t
                    ch = ctx_sb[:, :, roff : roff + M]
                    ps = psump.tile([128, D], F32, tag="ps")
                    for e0, ew in ECHUNKS:
                        for d in range(10):
                            nc.tensor.matmul(
                                ps[:M, e0 : e0 + ew],
                                lhsT=ch[:, d, :],
                                rhs=wk_t[:, d, e0 : e0 + ew],
                                start=(d == 0),
                                stop=False,
                            )
                        # bias via K=1 matmul
                        nc.tensor.matmul(
                            ps[:M, e0 : e0 + ew],
                            lhsT=onesb16[0:1, :M],
                            rhs=wkb_sb[0:1, s, e0 : e0 + ew],
                            start=False,
                            stop=True,
                        )
                    k, p0 = divmod(roff, 128)
                    if p0 == 0:
                        # aligned: PSUM -> fp16 pred tile directly on ACT
                        nc.scalar.copy(pred_tiles[k][0:M, :], ps[:M, :])
                    else:
                        # PSUM -> fp16 staging, DMA-repack across partitions
                        stg = stagep.tile([128, D], FP16, tag="stg")
                        nc.scalar.copy(stg[:M, :], ps[:M, :])
                        n1 = min(M, 128 - p0)
                        nc.sync.dma_start(
                            pred_tiles[k][p0 : p0 + n1, :], stg[0:n1, :]
                        )
                        if M > n1:
                            nc.sync.dma_start(
                                pred_tiles[k + 1][0 : M - n1, :], stg[n1:M, :]
                            )

            # ---- phase 2: gather fp16 targets, fused fp16 dots, CE stats ----
            ench_ap = ench[:, :]
            for sg in range(NSG):
                dots_t = dotsp.tile([128, NDOT], F32, tag="dots")
                for ci, (goff, w) in enumerate(GCHUNKS):
                    gt = gathp.tile([128, 4, D], FP16, tag="gt")
                    pos0 = sg * IDX_PER_SG + goff * 128
                    nidx = w * 128
                    nc.gpsimd.dma_gather(
                        gt[:, :w, :],
                        ench_ap,
                        idx_sb[:, pos0 // 16 : (pos0 + nidx) // 16],
                        nidx,
                        nidx,
                        D,
                        queue_num=(sg * len(GCHUNKS) + ci) % NQ,
                    )
                    for j in range(w):
                        scr = scrp.tile([128, D], FP16, tag="scr")
                        g = goff + j
                        # fused dot: out = (gt * 1.0) * pred, accum = sum(out)
                        nc.vector.scalar_tensor_tensor(
                            scr[:, :],
                            gt[:, j, :],
                            1.0,
                            pred_tiles[sg][:, :],
                            op0=Alu.mult,
                            op1=Alu.mult,
                            accum_out=dots_t[:, g : g + 1],
                        )
                # CE statistics for this supergroup (batch transcendentals later)
                nc.vector.tensor_reduce(
                    negm_all[:, sg : sg + 1], dots_t[:, :], Ax.X, Alu.max, negate=True
                )
                nc.vector.tensor_reduce(
                    mneg_all[:, sg : sg + 1], dots_t[:, 1:NDOT], Ax.X, Alu.max
                )
                nc.scalar.copy(pos_all[:, sg : sg + 1], dots_t[:, 0:1])
                e_t = scrp.tile([128, NDOT], F32, tag="et")
                nc.scalar.activation(
                    e_t[:, :],
                    dots_t[:, :],
                    Act.Exp,
                    bias=negm_all[:, sg : sg + 1],
                    scale=1.0,
                    accum_out=ssum_all[:, sg : sg + 1],
                )

            # ---- final: CE + accuracy over all supergroups at once ----
            lns = smallp.tile([128, NSG], F32, tag="lns")
            nc.scalar.activation(lns[:, :], ssum_all[:, :], Act.Ln)
            # loss = ln(sum) + m - pos  (negm = -m)
            t1 = smallp.tile([128, NSG], F32, tag="t1")
            nc.vector.tensor_tensor(t1[:, :], lns[:, :], negm_all[:, :], Alu.subtract)
            lossr = smallp.tile([128, NSG], F32, tag="lossr")
            nc.vector.tensor_tensor(lossr[:, :], t1[:, :], pos_all[:, :], Alu.subtract)
            corr = smallp.tile([128, NSG], F32, tag="corr")
            nc.vector.tensor_tensor(corr[:, :], pos_all[:, :], mneg_all[:, :], Alu.is_ge)
            lossm = smallp.tile([128, NSG], F32, tag="lossm")
            nc.vector.tensor_tensor(lossm[:, :], lossr[:, :], vmask[:, :], Alu.mult)
            corrm = smallp.tile([128, NSG], F32, tag="corrm")
            nc.vector.tensor_tensor(corrm[:, :], corr[:, :], vmask[:, :], Alu.mult)
            acc2 = smallp.tile([128, 2], F32, tag="acc2")
            nc.vector.tensor_reduce(acc2[:, 0:1], lossm[:, :], Ax.X, Alu.add)
            nc.vector.tensor_reduce(acc2[:, 1:2], corrm[:, :], Ax.X, Alu.add)

            # ---- final partition reduce: [128,2] -> [1,2] ----
            psf = psumfp.tile([1, 2], F32, tag="psf")
            nc.tensor.matmul(
                psf[:, :], lhsT=ones_sb[:, 0:1], rhs=acc2[:, :], start=True, stop=True
            )
            outsb = smallp.tile([1, 2], F32, tag="outsb")
            nc.vector.tensor_copy(outsb[:, :], psf[:, :])
            nc.sync.dma_start(out[:, :], outsb[:, :])

    nc.compile()
    return nc


def _row_targets(core: int, neg_idx: np.ndarray) -> np.ndarray:
    """[NR, 17] int array: flat enc index of positive + 16 negatives per row."""
    tg = np.zeros((NR, NDOT), np.int64)
    ri = 0
    for s in range(S):
        rows = 6 - s
        for b in range(BSH):
            bg = core * BSH + b
            for r in range(rows):
                for c7 in range(G):
                    tg[ri, 0] = bg * G * G + (s + 1 + r) * G + c7
                    tg[ri, 1:] = neg_idx[bg, s, r, c7]
                    ri += 1
    assert ri == NR
    return tg


def _build_idx(core: int, neg_idx: np.ndarray) -> np.ndarray:
    """int16 [128, IDX_TOT//16] gather-index tensor in SWDGE wrap layout."""
    tg = _row_targets(core, neg_idx)
    tg_pad = np.zeros((NSG * 128, NDOT), np.int64)
    tg_pad[:NR] = tg
    # list position sg*2176 + g*128 + p  ->  target of (row sg*128+p, dot g)
    lst = tg_pad.reshape(NSG, 128, NDOT).transpose(0, 2, 1).reshape(-1)
    arr = lst.astype(np.int16).reshape(-1, 16).T  # [16, IDX_TOT//16]
    return np.ascontiguousarray(np.tile(arr, (8, 1)))  # [128, ...]


def _prep_in_maps(contexts, encodings, Wk_w, Wk_b, neg_idx):
    contexts = np.ascontiguousarray(np.asarray(contexts, np.float32))
    encodings = np.ascontiguousarray(np.asarray(encodings, np.float32))
    Wk_w = np.ascontiguousarray(np.asarray(Wk_w, np.float32))
    Wk_b = np.ascontiguousarray(np.asarray(Wk_b, np.float32))
    neg_idx = np.asarray(neg_idx)

    ench = np.ascontiguousarray(encodings.reshape(B * G * G, D).astype(np.float16))
    # wk image: [S, 128 di, 10 do, 1280 e], di/do split of the contraction dim
    wkT = Wk_w.transpose(0, 2, 1).astype(ml_dtypes.bfloat16)  # [S, d, e]
    wkh = np.ascontiguousarray(wkT.reshape(S, 10, 128, D).transpose(0, 2, 1, 3))
    wkb = np.ascontiguousarray(Wk_b[None, :, :].astype(ml_dtypes.bfloat16))

    in_maps = []
    for c in range(NCORES):
        bs = slice(c * BSH, (c + 1) * BSH)
        ctx_rows = np.concatenate(
            [contexts[bs, : 6 - s].reshape(-1, D) for s in range(S)], axis=0
        )
        ctxT = ctx_rows.T.astype(ml_dtypes.bfloat16)  # [d, NR]
        ctxh = np.ascontiguousarray(ctxT.reshape(10, 128, NR).transpose(1, 0, 2))
        in_maps.append(
            {
                "ctxh": ctxh,
                "wkh": wkh,
                "wkb": wkb,
                "ench": ench,
                "idx": _build_idx(c, neg_idx),
            }
        )
    return in_maps


def kernel(contexts, encodings, Wk_w, Wk_b, neg_idx, _trace=False):
    in_maps = _prep_in_maps(contexts, encodings, Wk_w, Wk_b, neg_idx)
    nc = build_nc()
    res = run_bass_kernel_spmd(nc, in_maps, list(range(NCORES)), trace=_trace)
    LAST_RUN["exec_time_ns"] = res.exec_time_ns
    LAST_RUN["results"] = res.results
    loss = np.float32(0.0)
    corr = np.float32(0.0)
    for o in res.results:
        loss += np.float32(o["out"][0, 0])
        corr += np.float32(o["out"][0, 1])
    return (
        np.float32(loss / np.float32(N_PREDS)),
        np.float32(corr / np.float32(N_PREDS)),
    )


# revision 8
# speedup vs baseline: 1.5663x; 1.1544x over previous
"""Trainium2 Bass kernel for the CPC loss problem (nn_CPC_85117661872355).

Strategy (data-parallel over batch B across 8 cores), all-dense design:
  - Each core handles 8 of the 64 batch elements: 1120 prediction rows.
  - Phase 1 computes pred^T = Wk[s] @ ctx^T + b directly in transposed
    [e, row] layout on the PE (single bf16 pass, fp32 PSUM accumulate;
    stationary = Wk 128x128 chunk, streamed = ctx rows).  The bias is
    folded into the PSUM->SBUF evacuation as a per-partition ACT bias,
    and the fp16 pred^T layout is exactly what phase 2 wants as the
    stationary operand — no transposes, no staging DMAs.
  - Phase 2 computes ALL 3136 dots pred_row . enc_j per row as a dense
    PE matmul [128 rows x 3136] per supergroup (fp16 x fp16).  The 17
    logits per row (1 positive + 16 negatives) are extracted with three
    host-built fp16 mask tensors per supergroup:
      maskP: one-hot of the positive column  -> pos = sum(dots * P)
      maskW: multiplicity counts (incl. pos) -> ssum = sum(W * exp(.))
      maskB: 0 on selected columns else -3e4  -> m = rowmax(dots + B)
    All big DVE passes are fp16-packed (2x mode); row-reductions use a
    [128, 2, 1568] split so the reduce output keeps 2x eligibility.
    Ties between a duplicated negative and the positive stay exact
    (same dense matrix entry), matching jnp.argmax's first-index rule;
    accuracy is corr = (pos >= rowmax(selected)), identical to
    argmax==0.
  - Softmax-CE transcendentals are batched: Exp once per supergroup,
    one Ln over the [128, 9] sum-exp array at the end (3 ACT table
    loads total).  Pad rows (1120..1151) are given a synthetic logit
    so every lane stays finite; a validity mask zeroes them before the
    final reduction.
  - Per-core (loss_sum, correct_sum) are reduced over partitions with
    a K=128 ones-matmul and DMA'd out as [1,2]; host sums the 8 pairs.
"""

import functools

import ml_dtypes
import numpy as np

import concourse.bass as bass
import concourse.mybir as mybir
import concourse.tile as tile
from concourse import bacc
from concourse.bass_utils import run_bass_kernel_spmd

F32 = mybir.dt.float32
BF16 = mybir.dt.bfloat16
FP16 = mybir.dt.float16

B, G, D = 64, 7, 1280
S, NEG = 5, 16
NCORES = 8
BSH = B // NCORES  # 8
NS = [BSH * (6 - s) * G for s in range(S)]  # [336, 280, 224, 168, 112]
SOFF = [0]
for n in NS:
    SOFF.append(SOFF[-1] + n)
NR = SOFF[-1]  # 1120 rows per core
NSG = 9  # supergroups of 128 rows
NE = B * G * G  # 3136 encoding vectors
JCH = 448  # phase-2 column chunk (3136 = 7 * 448, one PSUM bank each)
N_PREDS = B * G * 20  # 8960
NEG_BIG = -30000.0

# Results of the last device run (for test harness introspection)
LAST_RUN = {}


@functools.lru_cache(maxsize=1)
def build_nc() -> bass.Bass:
    nc = bacc.Bacc(
        "TRN2",
        target_bir_lowering=False,
        debug=False,
        num_devices=NCORES,
    )
    # pre-shuffled SBUF images: [partition, ...contiguous per partition]
    ctxh = nc.declare_dram_parameter("ctxh", [128, 10, NR], BF16, isOutput=False)
    wkh = nc.declare_dram_parameter("wkh", [S, 128, 10, D], BF16, isOutput=False)
    wkbT = nc.declare_dram_parameter("wkbT", [128, S, 10], F32, isOutput=False)
    encTh = nc.declare_dram_parameter("encTh", [128, 10, NE], FP16, isOutput=False)
    maskW = nc.declare_dram_parameter("maskW", [NSG, 128, NE], FP16, isOutput=False)
    maskP = nc.declare_dram_parameter("maskP", [NSG, 128, NE], FP16, isOutput=False)
    maskB = nc.declare_dram_parameter("maskB", [NSG, 128, NE], FP16, isOutput=False)
    out = nc.declare_dram_parameter("out", [1, 2], F32, isOutput=True)

    Alu = mybir.AluOpType
    Act = mybir.ActivationFunctionType
    Ax = mybir.AxisListType

    with tile.TileContext(nc) as tc:
        with (
            tc.tile_pool(name="const", bufs=1) as constp,
            tc.tile_pool(name="mask", bufs=2) as maskp,
            tc.tile_pool(name="small", bufs=4) as smallp,
            tc.tile_pool(name="psumf", bufs=1, space="PSUM") as psumfp,
        ):
            # ---- persistent constants / stat arrays ----
            ones_sb = constp.tile([128, 1], F32, tag="ones")
            nc.vector.memset(ones_sb[:, :], 1.0)
            wkb_sb = constp.tile([128, S, 10], F32, tag="wkb")
            nc.sync.dma_start(wkb_sb[:, :, :], wkbT[:, :, :])
            m_all = constp.tile([128, NSG], F32, tag="mall")
            negm_all = constp.tile([128, NSG], F32, tag="negm")
            pos_all = constp.tile([128, NSG], F32, tag="pos")
            ssum_all = constp.tile([128, NSG], F32, tag="ssum")
            corr_all = constp.tile([128, NSG], F32, tag="corr")
            vmask = constp.tile([128, NSG], F32, tag="vmask")
            nc.vector.memset(vmask[:, :], 1.0)
            nc.vector.memset(vmask[96:128, NSG - 1 : NSG], 0.0)

            # resident enc^T fp16 image and pred^T output of phase 1
            # (allocated here; DMA emitted late so ctx/wk win the queue FIFO)
            encT_sb = constp.tile([128, 10, NE], FP16, tag="encT")
            predT_sb = constp.tile([128, 10, NR], FP16, tag="predT")

            # ---- phase 1: pred^T = Wk @ ctx^T + b (single bf16 pass) ----
            with (
                tc.tile_pool(name="p1", bufs=2) as p1p,
                tc.tile_pool(name="ps1", bufs=4, space="PSUM") as ps1p,
            ):
                ctx_sb = p1p.tile([128, 10, NR], BF16, tag="ctx", bufs=1)
                nc.sync.dma_start(ctx_sb[:, :, :], ctxh[:, :, :])
                for s in range(S):
                    wk_t = p1p.tile([128, 10, D], BF16, tag="wk", name=f"wk{s}")
                    nc.sync.dma_start(wk_t[:, :, :], wkh[s, :, :, :])
                    r0, rn = SOFF[s], NS[s]
                    for ec in range(10):
                        ps = ps1p.tile([128, 336], F32, tag="ps")
                        for dc in range(10):
                            nc.tensor.matmul(
                                ps[:, :rn],
                                lhsT=wk_t[:, dc, ec * 128 : (ec + 1) * 128],
                                rhs=ctx_sb[:, dc, r0 : r0 + rn],
                                start=(dc == 0),
                                stop=(dc == 9),
                            )
                        # PSUM -> fp16 pred^T with per-partition bias on ACT
                        nc.scalar.activation(
                            predT_sb[:, ec, r0 : r0 + rn],
                            ps[:, :rn],
                            Act.Identity,
                            bias=wkb_sb[:, s, ec : ec + 1],
                            scale=1.0,
                        )
                # enc^T load in halves, queued behind the wk loads
                nc.sync.dma_start(
                    encT_sb[:, :, 0 : NE // 2], encTh[:, :, 0 : NE // 2]
                )
                nc.sync.dma_start(
                    encT_sb[:, :, NE // 2 : NE], encTh[:, :, NE // 2 : NE]
                )

            # per-supergroup masks (rotating, prefetch up to 2 ahead)
            mB_t, mW_t, mP_t = [], [], []
            for sg in range(NSG):
                mB = maskp.tile([128, NE], FP16, tag="mB", name=f"mB{sg}")
                nc.sync.dma_start(mB[:, :], maskB[sg, :, :])
                mW = maskp.tile([128, NE], FP16, tag="mW", name=f"mW{sg}")
                nc.sync.dma_start(mW[:, :], maskW[sg, :, :])
                mP = maskp.tile([128, NE], FP16, tag="mP", name=f"mP{sg}")
                nc.sync.dma_start(mP[:, :], maskP[sg, :, :])
                mB_t.append(mB)
                mW_t.append(mW)
                mP_t.append(mP)

            # ---- phase 2: dense dots + masked softmax-CE per supergroup ----
            with (
                tc.tile_pool(name="p2", bufs=2) as p2p,
                tc.tile_pool(name="ps2", bufs=3, space="PSUM") as ps2p,
            ):
                for sg in range(NSG):
                    R = 128 if sg < 8 else 96
                    dots = p2p.tile([128, NE], FP16, tag="dots")
                    if sg == 8:
                        nc.vector.memset(dots[96:128, :], 0.0)
                    for jc in range(NE // JCH):
                        j0 = jc * JCH
                        ps2 = ps2p.tile([128, JCH], F32, tag="ps2")
                        for dc in range(10):
                            nc.tensor.matmul(
                                ps2[:R, :],
                                lhsT=predT_sb[:, dc, sg * 128 : sg * 128 + R],
                                rhs=encT_sb[:, dc, j0 : j0 + JCH],
                                start=(dc == 0),
                                stop=(dc == 9),
                            )
                        # PSUM -> fp16 dots on ACT
                        nc.scalar.copy(dots[:R, j0 : j0 + JCH], ps2[:R, :])

                    # masked = dots + maskB; m = rowmax(masked) via 2-col split
                    masked = p2p.tile([128, NE], FP16, tag="masked", bufs=1)
                    nc.vector.tensor_tensor(
                        masked[:, :], dots[:, :], mB_t[sg][:, :], Alu.add
                    )
                    mhalf = masked[:, :].rearrange("p (a b) -> p a b", a=2)
                    max2 = smallp.tile([128, 2], FP16, tag="max2")
                    nc.vector.tensor_reduce(max2[:, :], mhalf, Ax.X, Alu.max)
                    nc.vector.tensor_reduce(
                        m_all[:, sg : sg + 1], max2[:, :], Ax.X, Alu.max
                    )
                    nc.vector.tensor_reduce(
                        negm_all[:, sg : sg + 1], max2[:, :], Ax.X, Alu.max, negate=True
                    )
                    # pos = sum(dots * P)  (exact: single nonzero per row)
                    prod = p2p.tile([128, NE], FP16, tag="prod", bufs=1)
                    nc.vector.tensor_tensor(
                        prod[:, :], dots[:, :], mP_t[sg][:, :], Alu.mult
                    )
                    phalf = prod[:, :].rearrange("p (a b) -> p a b", a=2)
                    pos2 = smallp.tile([128, 2], FP16, tag="pos2")
                    # exact: one nonzero per row (host-validated)
                    with nc.allow_low_precision(reason="one-hot extract, exact"):
                        nc.vector.tensor_reduce(pos2[:, :], phalf, Ax.X, Alu.add)
                    nc.vector.tensor_reduce(
                        pos_all[:, sg : sg + 1], pos2[:, :], Ax.X, Alu.add
                    )
                    # E = exp(masked - m); ssum = sum(W * E)
                    e_t = p2p.tile([128, NE], FP16, tag="et", bufs=1)
                    nc.scalar.activation(
                        e_t[:, :],
                        masked[:, :],
                        Act.Exp,
                        bias=negm_all[:, sg : sg + 1],
                        scale=1.0,
                    )
                    prodw = p2p.tile([128, NE], FP16, tag="prodw", bufs=1)
                    nc.vector.tensor_tensor(
                        prodw[:, :], e_t[:, :], mW_t[sg][:, :], Alu.mult
                    )
                    whalf = prodw[:, :].rearrange("p (a b) -> p a b", a=2)
                    ssum2 = smallp.tile([128, 2], FP16, tag="ssum2")
                    # <=17 nonzero terms in [0,1]; host-validated ~6e-5 loss err
                    with nc.allow_low_precision(reason="sum-exp of 17 terms"):
                        nc.vector.tensor_reduce(ssum2[:, :], whalf, Ax.X, Alu.add)
                    nc.vector.tensor_reduce(
                        ssum_all[:, sg : sg + 1], ssum2[:, :], Ax.X, Alu.add
                    )
                    # corr = (pos >= rowmax of selected logits)
                    nc.vector.tensor_tensor(
                        corr_all[:, sg : sg + 1],
                        pos_all[:, sg : sg + 1],
                        m_all[:, sg : sg + 1],
                        Alu.is_ge,
                    )

                # ---- final: CE + accuracy over all supergroups at once ----
                lns = smallp.tile([128, NSG], F32, tag="lns")
                nc.scalar.activation(lns[:, :], ssum_all[:, :], Act.Ln)
                # loss = ln(sum) + m - pos
                t1 = smallp.tile([128, NSG], F32, tag="t1")
                nc.vector.tensor_tensor(t1[:, :], lns[:, :], m_all[:, :], Alu.add)
                lossr = smallp.tile([128, NSG], F32, tag="lossr")
                nc.vector.tensor_tensor(
                    lossr[:, :], t1[:, :], pos_all[:, :], Alu.subtract
                )
                lossm = smallp.tile([128, NSG], F32, tag="lossm")
                nc.vector.tensor_tensor(lossm[:, :], lossr[:, :], vmask[:, :], Alu.mult)
                corrm = smallp.tile([128, NSG], F32, tag="corrm")
                nc.vector.tensor_tensor(
                    corrm[:, :], corr_all[:, :], vmask[:, :], Alu.mult
                )
                acc2 = smallp.tile([128, 2], F32, tag="acc2")
                nc.vector.tensor_reduce(acc2[:, 0:1], lossm[:, :], Ax.X, Alu.add)
                nc.vector.tensor_reduce(acc2[:, 1:2], corrm[:, :], Ax.X, Alu.add)

                # final partition reduce: [128,2] -> [1,2]
                psf = psumfp.tile([1, 2], F32, tag="psf")
                nc.tensor.matmul(
                    psf[:, :],
                    lhsT=ones_sb[:, 0:1],
                    rhs=acc2[:, :],
                    start=True,
                    stop=True,
                )
                outsb = smallp.tile([1, 2], F32, tag="outsb")
                nc.vector.tensor_copy(outsb[:, :], psf[:, :])
                nc.sync.dma_start(out[:, :], outsb[:, :])

    nc.compile()
    return nc


def _row_targets(core: int, neg_idx: np.ndarray) -> np.ndarray:
    """[NR, 17] int array: flat enc index of positive + 16 negatives per row."""
    tg = np.zeros((NR, NEG + 1), np.int64)
    ri = 0
    for s in range(S):
        rows = 6 - s
        for b in range(BSH):
            bg = core * BSH + b
            for r in range(rows):
                for c7 in range(G):
                    tg[ri, 0] = bg * G * G + (s + 1 + r) * G + c7
                    tg[ri, 1:] = neg_idx[bg, s, r, c7]
                    ri += 1
    assert ri == NR
    return tg


def _build_masks(core: int, neg_idx: np.ndarray):
    """fp16 [NSG, 128, NE] maskW / maskP / maskB for this core."""
    tg = _row_targets(core, neg_idx)
    NPAD = NSG * 128
    rows = np.arange(NR)
    P = np.zeros((NPAD, NE), np.float32)
    P[rows, tg[:, 0]] = 1.0
    W = np.zeros((NPAD, NE), np.float32)
    np.add.at(W, (rows[:, None].repeat(NEG, 1).reshape(-1), tg[:, 1:].reshape(-1)), 1.0)
    W += P
    # pad rows: synthetic logit at column 0 keeps every lane finite
    P[NR:, 0] = 1.0
    W[NR:, 0] = 1.0
    Bm = np.where(W > 0, np.float32(0.0), np.float32(NEG_BIG))
    sh = (NSG, 128, NE)
    return (
        np.ascontiguousarray(W.reshape(sh).astype(np.float16)),
        np.ascontiguousarray(P.reshape(sh).astype(np.float16)),
        np.ascontiguousarray(Bm.reshape(sh).astype(np.float16)),
    )


def _prep_in_maps(contexts, encodings, Wk_w, Wk_b, neg_idx):
    contexts = np.ascontiguousarray(np.asarray(contexts, np.float32))
    encodings = np.ascontiguousarray(np.asarray(encodings, np.float32))
    Wk_w = np.ascontiguousarray(np.asarray(Wk_w, np.float32))
    Wk_b = np.ascontiguousarray(np.asarray(Wk_b, np.float32))
    neg_idx = np.asarray(neg_idx)

    # enc^T image: encTh[dp, dc, j] = enc_flat[j, dc*128+dp]
    enc_flat = encodings.reshape(NE, D).astype(np.float16)
    encTh = np.ascontiguousarray(enc_flat.T.reshape(10, 128, NE).transpose(1, 0, 2))
    # wk image: wkh[s, di, do, e] = WkT[s, do*128+di, e]
    wkT = Wk_w.transpose(0, 2, 1).astype(ml_dtypes.bfloat16)  # [S, d, e]
    wkh = np.ascontiguousarray(wkT.reshape(S, 10, 128, D).transpose(0, 2, 1, 3))
    # bias image: wkbT[p, s, ec] = Wk_b[s, ec*128+p]
    wkbT = np.ascontiguousarray(Wk_b.reshape(S, 10, 128).transpose(2, 0, 1))

    in_maps = []
    for c in range(NCORES):
        bs = slice(c * BSH, (c + 1) * BSH)
        ctx_rows = np.concatenate(
            [contexts[bs, : 6 - s].reshape(-1, D) for s in range(S)], axis=0
        )
        ctxT = ctx_rows.T.astype(ml_dtypes.bfloat16)  # [d, NR]
        ctxh = np.ascontiguousarray(ctxT.reshape(10, 128, NR).transpose(1, 0, 2))
        mW, mP, mB = _build_masks(c, neg_idx)
        in_maps.append(
            {
                "ctxh": ctxh,
                "wkh": wkh,
                "wkbT": wkbT,
                "encTh": encTh,
                "maskW": mW,
                "maskP": mP,
                "maskB": mB,
            }
        )
    return in_maps


def kernel(contexts, encodings, Wk_w, Wk_b, neg_idx, _trace=False):
    in_maps = _prep_in_maps(contexts, encodings, Wk_w, Wk_b, neg_idx)
    nc = build_nc()
    res = run_bass_kernel_spmd(nc, in_maps, list(range(NCORES)), trace=_trace)
    LAST_RUN["exec_time_ns"] = res.exec_time_ns
    LAST_RUN["results"] = res.results
    loss = np.float32(0.0)
    corr = np.float32(0.0)
    for o in res.results:
        loss += np.float32(o["out"][0, 0])
        corr += np.float32(o["out"][0, 1])
    return (
        np.float32(loss / np.float32(N_PREDS)),
        np.float32(corr / np.float32(N_PREDS)),
    )
